# Initial kernel scaffold
#
"""FINN Burgers solver (nn_FINN_Burger) as a Trainium2 Bass kernel.

Reference computation (per Euler step, 15 steps):
    a    = tanh(tanh(tanh(u @ W1) @ W2) @ W3)          # per-point scalar MLP
    flux = (u_left - u)*(D + relu(a)/DX) + (u_right - u)*(D - min(a,0)/DX)
    u   += dt * flux        (Dirichlet BC: u[-1] = u[Nx] = 0)

Sharding: Nx=16384 split across 8 cores (2048 points each) with a 32-point
ghost zone on each side.  15 steps only need a 15-point halo, so each core
integrates its 2112-point slab fully locally -- zero inter-core traffic.
Out-of-domain ghost points are forced to 0 each step via a mask input,
which also implements the Dirichlet boundary for cores 0 and 7.

Per-core layouts:
  u_row [1, 2112]  : MLP-facing row (point j at free offset j)
  u_ext [64, 35]   : flux-facing; partition p holds points [33p, 33p+33)
                     in cols 1..33, plus neighbour edge points in cols 0/34
  h1/h2 [128,2112] x4 chunks: hidden dim on partitions, points on free axis

MLP on device: PE broadcasts u to 128 partitions (ones-matmul), ACT fuses the
first-layer multiply into tanh via its per-partition scale operand, PE does
the 512x512 matmul (K-chunked accumulation in PSUM) and the 512->1 output
layer (M=1 matmuls), DVE evaluates the flux stencil, two small reshape DMAs
convert between row and 2-D layouts.
"""

import numpy as np

import concourse.bass as bass
import concourse.mybir as mybir
from concourse import tile
from concourse.bass_utils import run_bass_kernel_spmd

F32 = mybir.dt.float32
AF = mybir.ActivationFunctionType
OP = mybir.AluOpType

NX, H, NT = 16384, 512, 16
NCORES = 8
OWN = NX // NCORES          # 2048 points owned per core
P2D, B2D = 64, 33           # flux layout: 64 partitions x 33 points
NP = P2D * B2D              # 2112 slab points per core
GH = (NP - OWN) // 2        # 32-point ghost zone per side (need >= 15)
NSTEP = NT - 1
DX = 0.01
D_COEF = 0.01
# point-axis chunks for the matmuls (moving free dim <= 512)
CH = [(o, min(512, NP - o)) for o in range(0, NP, 512)]


def _build_nc():
    nc = bass.Bass("TRN2", target_bir_lowering=False, debug=False)

    u0s = nc.dram_tensor("u0slab", [1, NP], F32, kind="ExternalInput")
    w1d = nc.dram_tensor("w1", [1, H], F32, kind="ExternalInput")
    w2d = nc.dram_tensor("w2", [H, H], F32, kind="ExternalInput")
    w3d = nc.dram_tensor("w3", [H, 1], F32, kind="ExternalInput")
    tbd = nc.dram_tensor("tb", [128, NT], F32, kind="ExternalInput")
    mkd = nc.dram_tensor("mask", [P2D, B2D], F32, kind="ExternalInput")
    outd = nc.dram_tensor("out", [NT, OWN], F32, kind="ExternalOutput")

    with tile.TileContext(nc) as tc:
        with (
            tc.tile_pool(name="pers", bufs=1) as pers,
            tc.tile_pool(name="tmp", bufs=2) as tmp,
            tc.tile_pool(name="ps_ubc", bufs=1, space="PSUM") as ps_ubc,
            tc.tile_pool(name="ps_h2", bufs=2, space="PSUM") as ps_h2,
            tc.tile_pool(name="ps_a", bufs=1, space="PSUM") as ps_a,
        ):
            # ---- persistent tiles ----
            w2sb = [pers.tile([128, H], F32, name=f"w2sb{k}") for k in range(4)]
            w1t = pers.tile([128, 4], F32, name="w1t")    # w1t[p,j] = W1[0, 128j+p]
            w3t = pers.tile([128, 4], F32, name="w3t")    # w3t[p,k] = W3[128k+p, 0]
            ones = pers.tile([1, 128], F32, name="ones")
            tsb = pers.tile([128, NT], F32, name="tsb")
            dts = pers.tile([128, NSTEP], F32, name="dts")
            msk = pers.tile([P2D, B2D], F32, name="msk")
            u_ext = pers.tile([P2D, B2D + 2], F32, name="u_ext")
            u_row = pers.tile([1, NP], F32, name="u_row")
            a_row = pers.tile([1, NP], F32, name="a_row")
            h1 = [pers.tile([128, NP], F32, name=f"h1_{k}") for k in range(4)]
            h2 = [pers.tile([128, NP], F32, name=f"h2_{k}") for k in range(4)]

            # ---- init ----
            for k in range(4):
                nc.sync.dma_start(
                    out=w2sb[k][:, :], in_=w2d.ap()[128 * k : 128 * (k + 1), :]
                )
            nc.sync.dma_start(
                out=w1t[:, :], in_=w1d.ap().rearrange("a (c p) -> p (a c)", p=128)
            )
            nc.sync.dma_start(
                out=w3t[:, :], in_=w3d.ap().rearrange("(c p) a -> p (c a)", p=128)
            )
            nc.vector.memset(ones[:, :], 1.0)
            nc.sync.dma_start(out=tsb[:, :], in_=tbd.ap())
            nc.vector.tensor_sub(dts[:, :], tsb[:, 1:NT], tsb[:, 0 : NT - 1])
            nc.sync.dma_start(out=msk[:, :], in_=mkd.ap())
            nc.sync.dma_start(out=u_row[:, :], in_=u0s.ap())
            nc.vector.memset(u_ext[:, :], 0.0)
            nc.sync.dma_start(
                out=u_ext[:, 1 : 1 + B2D],
                in_=u0s.ap().rearrange("a (p c) -> p (a c)", p=P2D),
            )
            # step 0 output = u0
            nc.sync.dma_start(out=outd.ap()[0:1, :], in_=u_row[0:1, GH : GH + OWN])

            # ---- time steps ----
            for s in range(NSTEP):
                # broadcast u_row across 128 partitions (PSUM)
                ubc = ps_ubc.tile([128, NP], F32, name="ubc")
                for o, n in CH:
                    nc.tensor.matmul(
                        out=ubc[:, o : o + n],
                        lhsT=ones[0:1, :],
                        rhs=u_row[0:1, o : o + n],
                        start=True,
                        stop=True,
                    )
                # layer 1: h1[j] = tanh(W1[j] * u)   (scale operand = W1 column)
                for j in range(4):
                    nc.scalar.activation(
                        out=h1[j][:, :],
                        in_=ubc[:, :],
                        func=AF.Tanh,
                        scale=w1t[:, j : j + 1],
                    )
                # layer 2: h2[j] = tanh(sum_k W2[k,j]^T h1[k])
                for j in range(4):
                    for o, n in CH:
                        hp = ps_h2.tile([128, 512], F32, name="hp")
                        for k in range(4):
                            nc.tensor.matmul(
                                out=hp[:, :n],
                                lhsT=w2sb[k][:, 128 * j : 128 * (j + 1)],
                                rhs=h1[k][:, o : o + n],
                                start=(k == 0),
                                stop=(k == 3),
                            )
                        nc.scalar.activation(
                            out=h2[j][:, o : o + n], in_=hp[:, :n], func=AF.Tanh
                        )
                # layer 3: a = tanh(sum_k W3[k] h2[k])  (M=1 matmuls)
                for o, n in CH:
                    apr = ps_a.tile([1, 512], F32, name="apr")
                    for k in range(4):
                        nc.tensor.matmul(
                            out=apr[0:1, :n],
                            lhsT=w3t[:, k : k + 1],
                            rhs=h2[k][:, o : o + n],
                            start=(k == 0),
                            stop=(k == 3),
                        )
                    nc.scalar.activation(
                        out=a_row[0:1, o : o + n], in_=apr[0:1, :n], func=AF.Tanh
                    )

                # reshape a to the 2-D flux layout
                a2d = tmp.tile([P2D, B2D], F32, name="a2d")
                nc.sync.dma_start(out=a2d[:, :], in_=a_row[0:1, :])

                # neighbour edge columns (read pre-update u_ext)
                nc.sync.dma_start(
                    out=u_ext[1:P2D, 0:1], in_=u_ext[0 : P2D - 1, B2D : B2D + 1]
                )
                nc.sync.dma_start(
                    out=u_ext[0 : P2D - 1, B2D + 1 : B2D + 2], in_=u_ext[1:P2D, 1:2]
                )

                # flux stencil + Euler update (DVE)
                uC = u_ext[:, 1 : 1 + B2D]
                dul = tmp.tile([P2D, B2D], F32, name="dul")
                dur = tmp.tile([P2D, B2D], F32, name="dur")
                clp = tmp.tile([P2D, B2D], F32, name="clp")
                crp = tmp.tile([P2D, B2D], F32, name="crp")
                f1 = tmp.tile([P2D, B2D], F32, name="f1")
                f2 = tmp.tile([P2D, B2D], F32, name="f2")
                lap = tmp.tile([P2D, B2D], F32, name="lap")
                ft = tmp.tile([P2D, B2D], F32, name="ft")
                ft2 = tmp.tile([P2D, B2D], F32, name="ft2")
                upl = tmp.tile([P2D, B2D], F32, name="upl")

                nc.vector.tensor_sub(dul[:, :], u_ext[:, 0:B2D], uC)
                nc.vector.tensor_sub(dur[:, :], u_ext[:, 2 : 2 + B2D], uC)
                # clp = relu(a)/DX ; crp = -min(a,0)/DX  (both >= 0)
                nc.vector.tensor_scalar(
                    out=clp[:, :], in0=a2d[:, :], scalar1=0.0, scalar2=1.0 / DX,
                    op0=OP.max, op1=OP.mult,
                )
                nc.vector.tensor_scalar(
                    out=crp[:, :], in0=a2d[:, :], scalar1=0.0, scalar2=-1.0 / DX,
                    op0=OP.min, op1=OP.mult,
                )
                nc.vector.tensor_mul(f1[:, :], dul[:, :], clp[:, :])
                nc.vector.tensor_mul(f2[:, :], dur[:, :], crp[:, :])
                nc.vector.tensor_add(lap[:, :], dul[:, :], dur[:, :])
                # ft = lap*D + f1 ; ft2 = ft + f2  -> total flux
                nc.vector.scalar_tensor_tensor(
                    out=ft[:, :], in0=lap[:, :], scalar=D_COEF, in1=f1[:, :],
                    op0=OP.mult, op1=OP.add,
                )
                nc.vector.tensor_add(ft2[:, :], ft[:, :], f2[:, :])
                # u <- (u + dt*flux) * mask
                nc.vector.scalar_tensor_tensor(
                    out=upl[:, :], in0=ft2[:, :], scalar=dts[0:P2D, s : s + 1],
                    in1=uC, op0=OP.mult, op1=OP.add,
                )
                nc.vector.tensor_mul(uC, upl[:, :], msk[:, :])

                # back to row layout + store this step's owned slice
                nc.sync.dma_start(out=u_row[0:1, :], in_=uC)
                nc.sync.dma_start(
                    out=outd.ap()[s + 1 : s + 2, :], in_=u_row[0:1, GH : GH + OWN]
                )

    return nc


_NC_CACHE = {}


def _get_nc():
    if "nc" not in _NC_CACHE:
        _NC_CACHE["nc"] = _build_nc()
    return _NC_CACHE["nc"]


def _make_in_maps(t, u0, W1, W2, W3):
    t = np.asarray(t, np.float32)
    u0 = np.asarray(u0, np.float32).reshape(NX)
    W1 = np.ascontiguousarray(np.asarray(W1, np.float32).reshape(1, H))
    W2 = np.ascontiguousarray(np.asarray(W2, np.float32).reshape(H, H))
    W3 = np.ascontiguousarray(np.asarray(W3, np.float32).reshape(H, 1))
    tb = np.ascontiguousarray(np.broadcast_to(t.reshape(1, NT), (128, NT)))

    padded = np.zeros(NX + 2 * GH, np.float32)
    padded[GH : GH + NX] = u0

    in_maps = []
    for c in range(NCORES):
        slab = np.ascontiguousarray(padded[c * OWN : c * OWN + NP].reshape(1, NP))
        # mask over the slab (1 = inside the global domain): global index of
        # slab position j is c*OWN - GH + j ; positions p*B2D + c2 in 2-D layout
        gidx = c * OWN - GH + np.arange(NP)
        mask = ((gidx >= 0) & (gidx < NX)).astype(np.float32).reshape(P2D, B2D)
        in_maps.append(
            {
                "u0slab": slab,
                "w1": W1,
                "w2": W2,
                "w3": W3,
                "tb": tb,
                "mask": np.ascontiguousarray(mask),
            }
        )
    return in_maps


def _run(t, u0, W1, W2, W3, trace=False):
    nc = _get_nc()
    in_maps = _make_in_maps(t, u0, W1, W2, W3)
    res = run_bass_kernel_spmd(
        nc, in_maps, core_ids=list(range(NCORES)), trace=trace,
        trace_cores=list(range(NCORES)) if trace else None,
    )
    parts = [res.results[c]["out"] for c in range(NCORES)]
    full = np.concatenate(parts, axis=1).reshape(NT, NX, 1).astype(np.float32)
    return full, res


def kernel(t, u0, W1, W2, W3):
    full, _ = _run(t, u0, W1, W2, W3, trace=False)
    return full


# revision 8
# speedup vs baseline: 1.5141x; 1.5141x over previous
"""FINN Burgers solver (nn_FINN_Burger) as a Trainium2 Bass kernel.

Reference computation (per Euler step, 15 steps):
    a    = tanh(tanh(tanh(u @ W1) @ W2) @ W3)          # per-point scalar MLP
    flux = (u_left - u)*(D + relu(a)/DX) + (u_right - u)*(D - min(a,0)/DX)
    u   += dt * flux        (Dirichlet BC: u[-1] = u[Nx] = 0)

Sharding: Nx=16384 split across 8 cores (2048 points each) with a 32-point
ghost zone on each side.  15 steps only need a 15-point halo, so each core
integrates its 2112-point slab fully locally -- zero inter-core traffic.
Out-of-domain ghost points are forced to 0 each step via a mask input,
which also implements the Dirichlet boundary for cores 0 and 7.

Per-core layouts:
  u_row_g [1, 2114] : MLP-facing row with 1 guard cell each side
                      (point j of the slab lives at free offset j+1)
  u_ext [64, 35]    : flux-facing; partition p holds points [33p-1, 33p+34)
                      i.e. its 33 points plus both stencil neighbours; built
                      from u_row_g by one overlapping-window DMA
  h1/h2 [128,2112] x4 chunks: hidden dim on partitions, points on free axis

MLP on device: PE broadcasts u to 128 partitions (ones-matmul), ACT fuses the
first-layer multiply into tanh via its per-partition scale operand, PE does
the 512x512 matmul (K-chunked accumulation in PSUM) and the 512->1 output
layer (M=1 matmuls), DVE evaluates the flux stencil, small reshape DMAs
convert between row and 2-D layouts.
"""

import dataclasses

import numpy as np

import concourse.bacc as bacc
import concourse.bass as bass
import concourse.mybir as mybir
from concourse import tile
from concourse.bass_utils import run_bass_kernel_spmd

F32 = mybir.dt.float32
AF = mybir.ActivationFunctionType
OP = mybir.AluOpType

NX, H, NT = 16384, 512, 16
NCORES = 8
OWN = NX // NCORES          # 2048 points owned per core
P2D, B2D = 64, 33           # flux layout: 64 partitions x 33 points
NP = P2D * B2D              # 2112 slab points per core
GH = (NP - OWN) // 2        # 32-point ghost zone per side (need >= 15)
NSTEP = NT - 1
DX = 0.01
D_COEF = 0.01
# point-axis chunks for the matmuls (moving free dim <= 512)
CH = [(o, min(512, NP - o)) for o in range(0, NP, 512)]


def _build_nc(nrep=1):
    # nrep > 1 repeats the whole 15-step integration (wrong results, used
    # only for wall-clock timing via T(nrep) deltas in bench.py)
    nc = bacc.Bacc("TRN2", target_bir_lowering=False, debug=False)

    u0g = nc.dram_tensor("u0g", [1, NP + 2], F32, kind="ExternalInput")
    w1d = nc.dram_tensor("w1", [1, H], F32, kind="ExternalInput")
    w2d = nc.dram_tensor("w2", [H, H], F32, kind="ExternalInput")
    w3d = nc.dram_tensor("w3", [H, 1], F32, kind="ExternalInput")
    tbd = nc.dram_tensor("tb", [128, NT], F32, kind="ExternalInput")
    mkd = nc.dram_tensor("mask", [P2D, B2D], F32, kind="ExternalInput")
    outd = nc.dram_tensor("out", [NT, OWN], F32, kind="ExternalOutput")

    with tile.TileContext(nc) as tc:
        with (
            tc.tile_pool(name="pers", bufs=1) as pers,
            tc.tile_pool(name="tmp", bufs=2) as tmp,
            tc.tile_pool(name="ps_ubc", bufs=1, space="PSUM") as ps_ubc,
            tc.tile_pool(name="ps_h2", bufs=2, space="PSUM") as ps_h2,
            tc.tile_pool(name="ps_a", bufs=1, space="PSUM") as ps_a,
        ):
            # ---- persistent tiles ----
            w2sb = [pers.tile([128, H], F32, name=f"w2sb{k}") for k in range(4)]
            w1t = pers.tile([128, 4], F32, name="w1t")    # w1t[p,j] = W1[0, 128j+p]
            w3t = pers.tile([128, 4], F32, name="w3t")    # w3t[p,k] = W3[128k+p, 0]
            ones = pers.tile([1, 128], F32, name="ones")
            tsb = pers.tile([128, NT], F32, name="tsb")
            dts = pers.tile([128, NSTEP], F32, name="dts")
            msk = pers.tile([P2D, B2D], F32, name="msk")
            u_ext = pers.tile([P2D, B2D + 2], F32, name="u_ext")
            u_row = pers.tile([1, NP + 2], F32, name="u_row")
            a_row = pers.tile([1, NP], F32, name="a_row")
            h1 = [pers.tile([128, NP], F32, name=f"h1_{k}") for k in range(4)]
            h2 = [pers.tile([128, NP], F32, name=f"h2_{k}") for k in range(4)]

            # overlapping-window view of u_row: [64 x 35] windows, stride 33
            row_ap = u_row[0:1, 0 : NP + 2]
            win_ap = dataclasses.replace(
                row_ap, ap=[list(row_ap.ap[0]), [B2D, P2D], [1, B2D + 2]]
            )

            # ---- init ----
            for k in range(4):
                nc.sync.dma_start(
                    out=w2sb[k][:, :], in_=w2d.ap()[128 * k : 128 * (k + 1), :]
                )
            nc.sync.dma_start(
                out=w1t[:, :], in_=w1d.ap().rearrange("a (c p) -> p (a c)", p=128)
            )
            nc.sync.dma_start(
                out=w3t[:, :], in_=w3d.ap().rearrange("(c p) a -> p (c a)", p=128)
            )
            nc.vector.memset(ones[:, :], 1.0)
            nc.sync.dma_start(out=tsb[:, :], in_=tbd.ap())
            nc.vector.tensor_sub(dts[:, :], tsb[:, 1:NT], tsb[:, 0 : NT - 1])
            nc.sync.dma_start(out=msk[:, :], in_=mkd.ap())
            nc.sync.dma_start(out=u_row[:, :], in_=u0g.ap())
            nc.gpsimd.dma_start(out=u_ext[:, :], in_=win_ap)
            # step 0 output = u0
            nc.gpsimd.dma_start(
                out=outd.ap()[0:1, :], in_=u_row[0:1, 1 + GH : 1 + GH + OWN]
            )

            # ---- time steps ----
            for s in [s for _ in range(nrep) for s in range(NSTEP)]:
                # broadcast u across 128 partitions (PSUM)
                ubc = ps_ubc.tile([128, NP], F32, name="ubc")
                for o, n in CH:
                    nc.tensor.matmul(
                        out=ubc[:, o : o + n],
                        lhsT=ones[0:1, :],
                        rhs=u_row[0:1, 1 + o : 1 + o + n],
                        start=True,
                        stop=True,
                    )
                # layer 1: h1[j] = tanh(W1[j] * u)   (scale operand = W1 column)
                for j in range(4):
                    nc.scalar.activation(
                        out=h1[j][:, :],
                        in_=ubc[:, :],
                        func=AF.Tanh,
                        scale=w1t[:, j : j + 1],
                    )
                # layer 2: h2[j] = tanh(sum_k W2[k,j]^T h1[k])
                for j in range(4):
                    for o, n in CH:
                        hp = ps_h2.tile([128, 512], F32, name="hp")
                        for k in range(4):
                            nc.tensor.matmul(
                                out=hp[:, :n],
                                lhsT=w2sb[k][:, 128 * j : 128 * (j + 1)],
                                rhs=h1[k][:, o : o + n],
                                start=(k == 0),
                                stop=(k == 3),
                            )
                        nc.scalar.activation(
                            out=h2[j][:, o : o + n], in_=hp[:, :n], func=AF.Tanh
                        )
                # layer 3: a = tanh(sum_k W3[k] h2[k])  (M=1 matmuls)
                for o, n in CH:
                    apr = ps_a.tile([1, 512], F32, name="apr")
                    for k in range(4):
                        nc.tensor.matmul(
                            out=apr[0:1, :n],
                            lhsT=w3t[:, k : k + 1],
                            rhs=h2[k][:, o : o + n],
                            start=(k == 0),
                            stop=(k == 3),
                        )
                    nc.scalar.activation(
                        out=a_row[0:1, o : o + n], in_=apr[0:1, :n], func=AF.Tanh
                    )

                # reshape a to the 2-D flux layout
                a2d = tmp.tile([P2D, B2D], F32, name="a2d")
                nc.gpsimd.dma_start(out=a2d[:, :], in_=a_row[0:1, :])

                # flux stencil + Euler update (DVE)
                uC = u_ext[:, 1 : 1 + B2D]
                dul = tmp.tile([P2D, B2D], F32, name="dul")
                dur = tmp.tile([P2D, B2D], F32, name="dur")
                clp = tmp.tile([P2D, B2D], F32, name="clp")
                crp = tmp.tile([P2D, B2D], F32, name="crp")
                f1 = tmp.tile([P2D, B2D], F32, name="f1")
                f2 = tmp.tile([P2D, B2D], F32, name="f2")
                lap = tmp.tile([P2D, B2D], F32, name="lap")
                ft = tmp.tile([P2D, B2D], F32, name="ft")
                ft2 = tmp.tile([P2D, B2D], F32, name="ft2")
                upl = tmp.tile([P2D, B2D], F32, name="upl")
                unew = tmp.tile([P2D, B2D], F32, name="unew")

                nc.vector.tensor_sub(dul[:, :], u_ext[:, 0:B2D], uC)
                nc.vector.tensor_sub(dur[:, :], u_ext[:, 2 : 2 + B2D], uC)
                # clp = relu(a)/DX ; crp = -min(a,0)/DX  (both >= 0)
                nc.vector.tensor_scalar(
                    out=clp[:, :], in0=a2d[:, :], scalar1=0.0, scalar2=1.0 / DX,
                    op0=OP.max, op1=OP.mult,
                )
                nc.vector.tensor_scalar(
                    out=crp[:, :], in0=a2d[:, :], scalar1=0.0, scalar2=-1.0 / DX,
                    op0=OP.min, op1=OP.mult,
                )
                nc.vector.tensor_mul(f1[:, :], dul[:, :], clp[:, :])
                nc.vector.tensor_mul(f2[:, :], dur[:, :], crp[:, :])
                nc.vector.tensor_add(lap[:, :], dul[:, :], dur[:, :])
                # ft = lap*D + f1 ; ft2 = ft + f2  -> total flux
                nc.vector.scalar_tensor_tensor(
                    out=ft[:, :], in0=lap[:, :], scalar=D_COEF, in1=f1[:, :],
                    op0=OP.mult, op1=OP.add,
                )
                nc.vector.tensor_add(ft2[:, :], ft[:, :], f2[:, :])
                # u <- (u + dt*flux) * mask
                nc.vector.scalar_tensor_tensor(
                    out=upl[:, :], in0=ft2[:, :], scalar=dts[0:P2D, s : s + 1],
                    in1=uC, op0=OP.mult, op1=OP.add,
                )
                nc.vector.tensor_mul(unew[:, :], upl[:, :], msk[:, :])

                # back to row layout; rebuild u_ext off the critical path
                nc.gpsimd.dma_start(out=u_row[0:1, 1 : 1 + NP], in_=unew[:, :])
                nc.gpsimd.dma_start(out=u_ext[:, :], in_=win_ap)
                nc.gpsimd.dma_start(
                    out=outd.ap()[s + 1 : s + 2, :],
                    in_=u_row[0:1, 1 + GH : 1 + GH + OWN],
                )

    nc.finalize()
    return nc


_NC_CACHE = {}


def _get_nc(nrep=1):
    if nrep not in _NC_CACHE:
        _NC_CACHE[nrep] = _build_nc(nrep)
    return _NC_CACHE[nrep]


def _make_in_maps(t, u0, W1, W2, W3):
    t = np.asarray(t, np.float32)
    u0 = np.asarray(u0, np.float32).reshape(NX)
    W1 = np.ascontiguousarray(np.asarray(W1, np.float32).reshape(1, H))
    W2 = np.ascontiguousarray(np.asarray(W2, np.float32).reshape(H, H))
    W3 = np.ascontiguousarray(np.asarray(W3, np.float32).reshape(H, 1))
    tb = np.ascontiguousarray(np.broadcast_to(t.reshape(1, NT), (128, NT)))

    padded = np.zeros(NX + 2 * (GH + 1), np.float32)
    padded[GH + 1 : GH + 1 + NX] = u0

    in_maps = []
    for c in range(NCORES):
        slab = np.ascontiguousarray(
            padded[c * OWN : c * OWN + NP + 2].reshape(1, NP + 2)
        )
        # mask over the slab (1 = inside the global domain): global index of
        # slab position j is c*OWN - GH + j ; 2-D layout position p*B2D + c2
        gidx = c * OWN - GH + np.arange(NP)
        mask = ((gidx >= 0) & (gidx < NX)).astype(np.float32).reshape(P2D, B2D)
        in_maps.append(
            {
                "u0g": slab,
                "w1": W1,
                "w2": W2,
                "w3": W3,
                "tb": tb,
                "mask": np.ascontiguousarray(mask),
            }
        )
    return in_maps


def _run(t, u0, W1, W2, W3, trace=False):
    nc = _get_nc()
    in_maps = _make_in_maps(t, u0, W1, W2, W3)
    res = run_bass_kernel_spmd(
        nc, in_maps, core_ids=list(range(NCORES)), trace=trace,
        trace_cores=list(range(NCORES)) if trace else None,
    )
    parts = [res.results[c]["out"] for c in range(NCORES)]
    full = np.concatenate(parts, axis=1).reshape(NT, NX, 1).astype(np.float32)
    return full, res


def kernel(t, u0, W1, W2, W3):
    full, _ = _run(t, u0, W1, W2, W3, trace=False)
    return full
